# revision 12
# baseline (speedup 1.0000x reference)
"""Trainium2 Bass kernel for CSOCRG attention.

Computes, for latent [B,N,D] and alpha [B,N]:
    r[i,j]     = |i-j| + 1e-4
    ap[b,i,j]  = (alpha[b,i] + alpha[b,j]) / 2
    K[b,i,j]   = r^(-ap) * exp(-r / 64)
    K          = K / (row_sum(K) + 1e-8)
    out[b]     = K[b] @ latent[b]

Sharding: 8 cores = 4 batches x 2 row-halves (2048 rows each). Per core,
a block-banded kernel: K decays like exp(-|i-j|/64) * r^-ap, so only
128x128 blocks with tile distance |ti - tj| <= 1 are kept (every element
with |i-j| <= 128 is exact; the dropped tail is ~5.7e-3 relative).
Per 512-row pass, 6 j-tile strips are computed only over their in-band
i-columns (<=384 wide):
    K = exp(-0.5 * (a_i + a_j) * ln(r)) * exp(-r/64)
with one scalar_tensor_tensor (a_i+a_j)*ln(r) on GpSimd, one ScalarE Exp
(f16), and one 2x-mode f16 DVE multiply by the exp(-r/64) Toeplitz band
(ln(r) and exp(-r/64) are host Toeplitz bands, [128 x 384] — the full
|i-j| <= 255 band). The 12 in-band (jt, i-subtile) blocks per pass feed
f16 matmuls: K^T blocks stationary, latent moving, accumulated in 4 PSUM
banks, plus 1-column mask matmuls in a fifth bank for row sums.
Normalization is a per-partition reciprocal scale split across ScalarE
and DVE, emitting f16 outputs (host upcasts to f32).
"""

import os
import sys
import numpy as np
from contextlib import ExitStack

for _p in (
    "/opt/trn_rl_repo",
    "/opt/trn_rl_repo/concourse",
    "/root/.axon_site/_ro/trn_rl_repo",
    "/root/.axon_site/_ro/trn_rl_repo/concourse",
):
    if os.path.isdir(_p) and _p not in sys.path:
        sys.path.append(_p)

# ---------------- problem constants (hardcoded per spec) ----------------
B, N, D = 4, 4096, 512
NCORES = 8
HALF = N // 2            # rows per core
PAD = 128                # j-window halo per side (one tile)
JW = (HALF + 2 * PAD) // 128             # j-tiles in the window (18)
WPASS = 512              # PSUM pass width (nt = WPASS/128 num banks + row)
NT = WPASS // 128
PASSES = list(range(0, HALF, WPASS))
G = 384                  # Toeplitz band width: offsets |i-j| <= 255
DMIN = -128
LAMBDA_RG = 64.0
EPS_R = 1e-4
EPS_SUM = 1e-8

_PROGRAM_CACHE = {}
last_exec_time_ns = None


def _split_multi_waits(nc, max_waits=1):
    """Cap sem-waits per instruction for this walrus build.

    The walrus here rejects instructions carrying multiple sync wait
    commands ("Too many sync wait commands"). Tile attaches one wait per
    producing proc. Splitting is safe: excess waits move onto NoOp
    carriers inserted immediately before the instruction on the same
    engine, so the engine stream blocks at the exact same point.
    """
    import mybir

    k = 0
    for fn in nc.m.functions:
        for bb in fn.blocks:
            new = []
            for inst in bb.instructions:
                si = inst.sync_info
                waits = list(si.on_wait) if si is not None and si.on_wait else []
                if len(waits) > max_waits:
                    keep = waits[:max_waits]
                    extra = waits[max_waits:]
                    for i in range(0, len(extra), max_waits):
                        k += 1
                        nop = mybir.InstNoOp(
                            name=f"wsplit-{k}", ins=[], outs=[])
                        nop.engine = inst.engine
                        nop.sync_info = mybir.SyncInfo(
                            on_wait=extra[i:i + max_waits], on_update=[])
                        nc.register_instruction(nop, overwrite=True)
                        new.append(nop)
                    inst.sync_info = mybir.SyncInfo(
                        on_wait=keep,
                        on_update=list(si.on_update) if si.on_update else [])
                new.append(inst)
            bb.instructions = new
    return nc


# per half-pass strip groups: (ks, per-strip offset in the 768-wide group)
GROUPS = (((-1, 0), (0, 128)), ((1, 2), (0, 384)), ((3, 4), (0, 256)))
GW = (384, 768, 384)


def _strip_geom(k):
    """(tlo, thi, lo, hi, wk, off) for strip k of any pass."""
    tlo, thi = max(0, k - 1), min(NT - 1, k + 1)
    lo, hi = 128 * tlo, 128 * (thi + 1)
    return tlo, thi, lo, hi, hi - lo, -128 * k - DMIN


# packed f16 constant layout: mcol | acol | lbig | ebig groups
C_MCOL = 0
C_ACOL = C_MCOL + JW
C_LBIG = C_ACOL + JW
C_EGRP = C_LBIG + G
CW = C_EGRP + sum(GW)
EGOFF = [C_EGRP + sum(GW[:g]) for g in range(len(GW))]


def build_program(repeat=1):
    from concourse import bass, tile
    import mybir

    f32 = mybir.dt.float32
    f16 = mybir.dt.float16
    ALU = mybir.AluOpType
    ACTF = mybir.ActivationFunctionType

    nc = bass.Bass()
    lat_d = nc.declare_dram_parameter(
        "latent_win", [JW * 128, D], f16, isOutput=False)
    cst_d = nc.declare_dram_parameter("consts", [128, CW], f16, isOutput=False)
    abc_d = nc.declare_dram_parameter("alpha_bcast", [128, HALF], f16, isOutput=False)
    out_d = nc.declare_dram_parameter("out", [HALF, D], f16, isOutput=True)

    with ExitStack() as ctx:
        tc = ctx.enter_context(tile.TileContext(nc))
        const = ctx.enter_context(tc.tile_pool(name="const", bufs=1))
        wp = ctx.enter_context(tc.tile_pool(name="wp", bufs=4))
        kp = ctx.enter_context(tc.tile_pool(name="kp", bufs=4))
        outp = ctx.enter_context(tc.tile_pool(name="outp", bufs=3))
        rp = ctx.enter_context(tc.tile_pool(name="rp", bufs=2))
        pp = ctx.enter_context(tc.tile_pool(name="pp", bufs=1, space="PSUM"))

        # ---- DMAs: one packed constant load + 3 latent chunks ----
        # consts (gates the first strips) first on the sync ring; latent
        # on the tensor-engine hardware DGE ring so dispatch overlaps
        cst = const.tile([128, CW], f16)
        abc = const.tile([128, HALF], f16)
        # sync ring: stt inputs first (mcol|acol|lbig head + abc chunk 0),
        # then the ebig tail (needed one pipeline stage later) + abc rest
        nc.sync.dma_start(cst[:, :C_EGRP], cst_d[:, :C_EGRP])
        nc.sync.dma_start(abc[:, :1024], abc_d[:, :1024])
        nc.sync.dma_start(cst[:, C_EGRP:], cst_d[:, C_EGRP:])
        nc.sync.dma_start(abc[:, 1024:], abc_d[:, 1024:])
        lat_view = lat_d.rearrange("(t p) d -> p t d", p=128)
        lat_tiles, lat_of = [], []
        for c0, cn in ((0, 2), (2, 8), (10, 8)):
            lt = const.tile([128, cn, D], f16, name=f"lat{c0}")
            nc.scalar.dma_start(lt[:], lat_view[:, c0:c0 + cn, :])
            lat_tiles.append(lt)
            lat_of.append(c0)

        def lat_sb_tile(jt):
            c = 0 if jt < 2 else (1 if jt < 10 else 2)
            return lat_tiles[c][:, jt - lat_of[c], :]

        def emit_passes():
          for i0 in PASSES:
            q = i0 // 128 + 1       # window tile index of the pass start
            # 4 num banks + 1 rowsum bank per pass; 3 spare banks double-
            # buffer num0-2 so the next pass's matmuls overlap this pass's
            # normalization reads
            nums = [pp.tile([128, D], f32, tag=f"num{t7}", name=f"num{t7}",
                            bufs=2 if t7 < 3 else 1)
                    for t7 in range(NT)]
            row = pp.tile([128, 16], f32, tag="row")
            for g, (ks, goffs) in enumerate(GROUPS):
                gw = GW[g]
                # w = (a_i + a_j) * ln(r) per strip  [128 part = j, free = i]
                w = wp.tile([128, 768], f16, tag="w")
                for k, go in zip(ks, goffs):
                    jt = q + k
                    tlo, thi, lo, hi, wk, off = _strip_geom(k)
                    nc.vector.scalar_tensor_tensor(
                        w[:, go:go + wk], abc[:, i0 + lo:i0 + hi],
                        cst[:, C_ACOL + jt:C_ACOL + jt + 1],
                        cst[:, C_LBIG + off + lo:C_LBIG + off + hi],
                        ALU.add, ALU.mult)
                # p = exp(-0.5*w); kt = p * exp(-r/64), one op per group
                p = kp.tile([128, 768], f16, tag="z")
                nc.scalar.activation(p[:, :gw], w[:, :gw], ACTF.Exp, scale=-0.5)
                kt = kp.tile([128, 768], f16, tag="k")
                nc.vector.tensor_mul(kt[:, :gw], p[:, :gw],
                                     cst[:, EGOFF[g]:EGOFF[g] + gw])
                for k, go in zip(ks, goffs):
                    jt = q + k
                    tlo, thi, lo, hi, wk, off = _strip_geom(k)
                    for t7 in range(tlo, thi + 1):
                        stat = kt[:, go + 128 * t7 - lo:go + 128 * t7 - lo + 128]
                        nc.tensor.matmul(
                            nums[t7][:], stat, lat_sb_tile(jt),
                            start=(k == t7 - 1), stop=(k == t7 + 1))
                        nc.tensor.matmul(
                            row[:, t7:t7 + 1], stat,
                            cst[:, C_MCOL + jt:C_MCOL + jt + 1],
                            start=(k == -1), stop=(k == NT))
            # normalize: out = num / (rowsum + 1e-8)
            rs = rp.tile([128, 8], f32, tag="rs")
            nc.vector.tensor_scalar_add(rs[:, :NT], row[:, :NT], EPS_SUM)
            rec = rp.tile([128, 8], f32, tag="rec")
            nc.vector.reciprocal(rec[:, :NT], rs[:, :NT])
            # copy the single-buffered banks (3, then 2) first so the next
            # pass's matmuls reclaim them sooner; DMA each half as it lands
            o = outp.tile([128, NT, D], f16, tag="o")
            for t7 in (3, 2, 1, 0):
                nc.scalar.activation(o[:, t7, :], nums[t7][:], ACTF.Copy,
                                     scale=rec[:, t7:t7 + 1])
                if t7 == 2:
                    nc.sync.dma_start(
                        out_d[i0 + 256:i0 + WPASS].rearrange(
                            "(t p) d -> p t d", p=128), o[:, 2:, :])
            nc.sync.dma_start(
                out_d[i0:i0 + 256].rearrange("(t p) d -> p t d", p=128),
                o[:, :2, :])

        if repeat > 1:
            # hardware loop over identical rounds — used only for timing
            with tc.For_i(0, repeat, 1, hint_engines=(
                    mybir.EngineType.PE, mybir.EngineType.DVE,
                    mybir.EngineType.Activation, mybir.EngineType.SP)):
                emit_passes()
        else:
            emit_passes()
    return _split_multi_waits(nc)


def host_inputs(latent, alpha):
    """Build the 8 per-core input maps."""
    latent = np.asarray(latent, dtype=np.float32)
    alpha = np.asarray(alpha, dtype=np.float32)
    d = (np.arange(G, dtype=np.int64)[None, :]
         - np.arange(128, dtype=np.int64)[:, None] + DMIN)
    ad = np.abs(d).astype(np.float32)
    lbig = np.log(ad + np.float32(EPS_R)).astype(np.float16)
    ebig = np.exp(-(ad + np.float32(EPS_R)) / np.float32(LAMBDA_RG))
    ebig = ebig.astype(np.float16)
    # ebig regrouped to the per-pass strip-group layout
    egrp = np.zeros((128, sum(GW)), np.float16)
    for g, (ks, goffs) in enumerate(GROUPS):
        for k, go in zip(ks, goffs):
            _, _, lo, hi, wk, off = _strip_geom(k)
            egrp[:, EGOFF[g] - C_EGRP + go:EGOFF[g] - C_EGRP + go + wk] = \
                ebig[:, off + lo:off + hi]

    in_maps = []
    for c in range(NCORES):
        b, h = c // 2, c % 2
        r0 = h * HALF
        jlo = r0 - PAD
        lo, hi = max(0, jlo), min(N, jlo + JW * 128)
        win = np.zeros((JW * 128, D), np.float16)
        win[lo - jlo: hi - jlo] = latent[b, lo:hi].astype(np.float16)
        aw = np.zeros(JW * 128, np.float16)
        aw[lo - jlo: hi - jlo] = alpha[b, lo:hi].astype(np.float16)
        mw = np.zeros(JW * 128, np.float16)
        mw[lo - jlo: hi - jlo] = 1.0
        cst = np.zeros((128, CW), np.float16)
        cst[:, C_MCOL:C_MCOL + JW] = mw.reshape(JW, 128).T
        cst[:, C_ACOL:C_ACOL + JW] = aw.reshape(JW, 128).T
        cst[:, C_LBIG:C_LBIG + G] = lbig
        cst[:, C_EGRP:] = egrp
        m = {
            "latent_win": win,
            "consts": cst,
            "alpha_bcast": np.ascontiguousarray(np.broadcast_to(
                alpha[b, r0:r0 + HALF][None, :], (128, HALF))).astype(np.float16),
        }
        in_maps.append(m)
    return in_maps


def _get_exec(repeat=1):
    """Build (once) a jitted 8-core shard_map executable for the program."""
    key = f"exec-blk-{repeat}"
    if key in _PROGRAM_CACHE:
        return _PROGRAM_CACHE[key]
    import jax
    from jax.sharding import Mesh, PartitionSpec
    from jax.experimental.shard_map import shard_map
    from concourse import bass2jax
    import mybir

    nc = build_program(repeat=repeat)
    bass2jax.install_neuronx_cc_hook()

    partition_name = (nc.partition_id_tensor.name
                      if nc.partition_id_tensor else None)
    in_names, out_names, out_avals = [], [], []
    for alloc in nc.m.functions[0].allocations:
        if not isinstance(alloc, mybir.MemoryLocationSet):
            continue
        name = alloc.memorylocations[0].name
        if alloc.kind == "ExternalInput":
            if name != partition_name:
                in_names.append(name)
        elif alloc.kind == "ExternalOutput":
            out_names.append(name)
            out_avals.append(jax.core.ShapedArray(
                tuple(alloc.tensor_shape), mybir.dt.np(alloc.dtype)))
    n_params = len(in_names)
    all_in = list(in_names) + list(out_names)
    if partition_name is not None:
        all_in.append(partition_name)
    all_in = tuple(all_in)
    donate = tuple(range(n_params, n_params + len(out_names)))

    def _body(*args):
        operands = list(args)
        if partition_name is not None:
            operands.append(bass2jax.partition_id_tensor())
        outs = bass2jax._bass_exec_p.bind(
            *operands,
            out_avals=tuple(out_avals),
            in_names=all_in,
            out_names=tuple(out_names),
            lowering_input_output_aliases=(),
            sim_require_finite=True,
            sim_require_nnan=True,
            nc=nc,
        )
        return tuple(outs)

    devices = jax.devices()[:NCORES]
    assert len(devices) == NCORES, f"need {NCORES} cores, have {len(jax.devices())}"
    mesh = Mesh(np.asarray(devices), ("core",))
    in_specs = (PartitionSpec("core"),) * (n_params + len(out_names))
    out_specs = (PartitionSpec("core"),) * len(out_names)
    sharded = jax.jit(
        shard_map(_body, mesh=mesh, in_specs=in_specs,
                  out_specs=out_specs, check_rep=False),
        donate_argnums=donate, keep_unused=True)
    _PROGRAM_CACHE[key] = (sharded, in_names, out_names, out_avals)
    return _PROGRAM_CACHE[key]


def _concat_inputs(in_maps, in_names):
    return [np.concatenate([in_maps[c][nm] for c in range(NCORES)], axis=0)
            for nm in in_names]


def _zeros_outs(out_avals):
    return [np.zeros((NCORES * av.shape[0], *av.shape[1:]), av.dtype)
            for av in out_avals]


def _gather_out(res):
    out = np.empty((B, N, D), np.float32)
    for c in range(NCORES):
        b, h = c // 2, c % 2
        out[b, h * HALF:(h + 1) * HALF] = res[c].astype(np.float32)
    return out


def kernel(latent, alpha):
    sharded, in_names, out_names, out_avals = _get_exec()
    in_maps = host_inputs(latent, alpha)
    outs = sharded(*_concat_inputs(in_maps, in_names), *_zeros_outs(out_avals))
    res = np.asarray(outs[out_names.index("out")]).reshape(NCORES, HALF, D)
    return _gather_out(res)


def timed_run(latent, alpha, iters=12, r_lo=64, r_hi=512):
    """Return (out, [estimated per-kernel device ns]).

    Device time is invisible in single-launch wall clock (~90ms RPC per
    launch, +-10ms noise), so the kernel body is looped r times on-device
    (tc.For_i) and the per-round time comes from the wall-time slope
    between two loop counts. The slope includes ~5-9us/round of loop
    back-edge overhead, so it slightly overestimates the plain kernel.
    """
    import time
    import jax
    sharded, in_names, out_names, out_avals = _get_exec()
    in_maps = host_inputs(latent, alpha)
    concat_in = _concat_inputs(in_maps, in_names)
    dev_in = [jax.device_put(a) for a in concat_in]
    jax.block_until_ready(dev_in)

    outs = sharded(*dev_in, *_zeros_outs(out_avals))
    jax.block_until_ready(outs)
    res = np.asarray(outs[out_names.index("out")]).reshape(NCORES, HALF, D)
    out = _gather_out(res)

    lo = _get_exec(r_lo)[0]
    hi = _get_exec(r_hi)[0]

    def one(fn):
        zs = [jax.device_put(z) for z in _zeros_outs(out_avals)]
        jax.block_until_ready(zs)
        t0 = time.perf_counter()
        o = fn(*dev_in, *zs)
        jax.block_until_ready(o)
        return time.perf_counter() - t0

    one(lo), one(hi)  # warm/compile
    tlo, thi = [], []
    for _ in range(iters):
        tlo.append(one(lo))
        thi.append(one(hi))
    med = lambda v: sorted(v)[len(v) // 2]
    est = (med(thi) - med(tlo)) / (r_hi - r_lo) * 1e9
    print(f"  r{r_lo} min/med: {min(tlo)*1e3:.1f}/{med(tlo)*1e3:.1f} ms"
          f"   r{r_hi} min/med: {min(thi)*1e3:.1f}/{med(thi)*1e3:.1f} ms")
    return out, [est]
